# revision 6
# baseline (speedup 1.0000x reference)
"""GCN layer (SpMM) Bass kernel for 8 trn2 NeuronCores.

out[i] = sum_{e: rows[e]==i} edge_vals[e] * embeds[cols[e]]
N=100000 nodes, E=1000000 edges, D=64 features.

Strategy: host sorts edges by destination row; nodes split into 8
contiguous ranges (12500/core, disjoint outputs -> no collectives).
Per core, output rows are processed in 128-row blocks. Edge embeddings
are fetched with batched SWDGE dma_gather (InstDMAGatherAnt): int16
indices limit a gather to a 32k-row window, so columns are partitioned
into 4 ranges of 25000 rows and each (block, range) group is padded to
whole chunks of 128 edges. Gathers run in 14-chunk segments
(1792 idxs, single_packet=False — >=1152 idxs with single_packet=True
kills the device — round-robin over 4 SWDGE queues, prefetched
LOOKAHEAD segments ahead of consumption). Per 16-chunk strip the DVE
builds val-scaled one-hots with two batched broadcast tensor_tensor ops
(bf16): oh[p, j, r] = (iota[r]==rrow[p,j]) * val[p,j]. Per 14-chunk
segment the scalar engine converts gathered embeddings fp32->bf16 in
one op. Per chunk, one bf16 matmul accumulates psum[r,:] += oh_j.T @
embs[p,:]. After a block's chunks, PSUM is copied to SBUF and DMA'd to
the contiguous output rows. Device time ~0.3 ms/exec (the gather is at
the random-256B HBM roofline ~140 GB/s/core); measured wall time is
dominated by the ~30-70 ms axon dispatch floor.
"""

import sys

import numpy as np

if "/opt/trn_rl_repo" not in sys.path:
    sys.path.insert(0, "/opt/trn_rl_repo")

N_NODES = 100000
D = 64
P = 128
N_CORES = 8
NR = 4  # column ranges (int16 gather index limit)
SEG = 15  # chunks per dma_gather (SWDGE ring limit: num_idxs/16+1 <= 128)


def _build_program(m, n_nodes, repeats=1, ablate=None):
    """m: [NBLK, NR] chunks per (block, range)."""
    import concourse.bacc as bacc
    import concourse.tile as tile
    from concourse import mybir

    npc = n_nodes // N_CORES
    nblk = (npc + P - 1) // P
    rb = -(-n_nodes // NR)  # range width
    Q = int(m.sum())
    # range-major chunk base: cbase[r] = start of range r's chunk span
    Mr = m.sum(axis=0)  # [NR]
    Cr = np.concatenate([[0], np.cumsum(Mr)]).astype(int)

    nc = bacc.Bacc(
        "TRN2",
        target_bir_lowering=False,
        debug=False,
        num_devices=N_CORES,
        num_swdge_queues=4,
    )
    # 4 separate range tensors: HW mis-applies large AP offsets on the
    # dma_gather source (sim honors them; >~256KB offsets read wrong data)
    emb_rt = [
        nc.dram_tensor(
            f"embeds_r{r}",
            [min(rb * (r + 1), n_nodes) - rb * r, D],
            mybir.dt.float32,
            kind="ExternalInput",
        )
        for r in range(NR)
    ]
    idx_t = nc.dram_tensor("idx_w", [P, Q * 8], mybir.dt.int16, kind="ExternalInput")
    vals_t = nc.dram_tensor("vals_p", [P, Q], mybir.dt.float32, kind="ExternalInput")
    rrow_t = nc.dram_tensor("rrow_p", [P, Q], mybir.dt.float32, kind="ExternalInput")
    iota_t = nc.dram_tensor("iota", [P, P], mybir.dt.bfloat16, kind="ExternalInput")
    out_t = nc.dram_tensor(
        "out", [nblk * P, D], mybir.dt.float32, kind="ExternalOutput"
    )

    import os as _os
    _ohb = int(_os.environ.get("OH_BUFS", "6"))
    _embb = int(_os.environ.get("EMB_BUFS", "5"))
    _ebfb = int(_os.environ.get("EBF_BUFS", "6"))
    with tile.TileContext(nc) as tc:
        with (
            tc.tile_pool(name="static", bufs=1) as static_pool,
            tc.tile_pool(name="emb0", bufs=_embb) as ep0,
            tc.tile_pool(name="emb1", bufs=_embb) as ep1,
            tc.tile_pool(name="emb2", bufs=_embb) as ep2,
            tc.tile_pool(name="emb3", bufs=_embb) as ep3,
            tc.tile_pool(name="ebf0", bufs=_ebfb) as bp0,
            tc.tile_pool(name="ebf1", bufs=_ebfb) as bp1,
            tc.tile_pool(name="ebf2", bufs=_ebfb) as bp2,
            tc.tile_pool(name="ebf3", bufs=_ebfb) as bp3,
            tc.tile_pool(name="ohv", bufs=_ohb) as ohv_pool,
            tc.tile_pool(name="outp", bufs=4) as out_pool,
            tc.tile_pool(name="psum", bufs=8, space="PSUM") as psum_pool,
        ):
            embp = [ep0, ep1, ep2, ep3]
            ebfp = [bp0, bp1, bp2, bp3]
            idx_sb = static_pool.tile([P, Q * 8], mybir.dt.int16)
            vals_sb = static_pool.tile([P, Q], mybir.dt.float32)
            rrow_sb = static_pool.tile([P, Q], mybir.dt.float32)
            iota_sb = static_pool.tile([P, P], mybir.dt.bfloat16)
            nc.sync.dma_start(out=idx_sb[:], in_=idx_t[:])
            nc.sync.dma_start(out=vals_sb[:], in_=vals_t[:])
            nc.sync.dma_start(out=rrow_sb[:], in_=rrow_t[:])
            nc.sync.dma_start(out=iota_sb[:], in_=iota_t[:])

            import os as _os
            LOOKAHEAD = int(_os.environ.get("LOOKAHEAD", "3"))
            for _rep in range(repeats):
              cursor = [int(Cr[r]) for r in range(NR)]
              seg_tiles = [dict() for _ in range(NR)]  # s -> (ebf, lo)
              next_emit = [0] * NR
              n_seg = [
                  -(-int(Mr[r]) // SEG) if int(Mr[r]) else 0 for r in range(NR)
              ]
              q = 0
              n_gather = 0

              def make_oh(q_):
                  # oh_val[p, r] = (iota[p, r] == rrow[p, q_]) * vals[p, q_]
                  # One fused DVE tensor_scalar per chunk: all operands are
                  # SBUF/2-byte/innermost-contiguous (scalars are per-
                  # partition APs, exempt) -> 4x DVE perf mode, vs the 1x
                  # broadcast tensor_tensor this replaces.
                  oh_val = ohv_pool.tile([P, P], mybir.dt.bfloat16)
                  nc.vector.tensor_scalar(
                      out=oh_val[:],
                      in0=iota_sb[:],
                      scalar1=rrow_sb[:, q_ : q_ + 1],
                      scalar2=vals_sb[:, q_ : q_ + 1],
                      op0=mybir.AluOpType.is_equal,
                      op1=mybir.AluOpType.mult,
                  )
                  return oh_val

              def emit_seg(r):
                  nonlocal n_gather
                  s_ = next_emit[r]
                  lo = int(Cr[r]) + s_ * SEG
                  hi = min(lo + SEG, int(Cr[r + 1]))
                  L = hi - lo
                  emb = embp[r].tile([P, L, D], mybir.dt.float32)
                  if ablate not in ("comp", "dve", "pe"):
                      nc.gpsimd.dma_gather(
                          emb[:],
                          emb_rt[r][:],
                          idx_sb[:, lo * 8 : hi * 8],
                          L * P,
                          L * P,
                          D,
                          single_packet=False,
                          queue_num=n_gather % 4,
                      )
                      n_gather += 1
                  ebf = ebfp[r].tile([P, L, D], mybir.dt.bfloat16)
                  if ablate not in ("gather", "comp", "dve", "pe"):
                      nc.scalar.copy(out=ebf[:], in_=emb[:])
                  elif ablate in ("comp", "dve", "pe"):
                      nc.gpsimd.memzero(ebf[:])
                  seg_tiles[r][s_] = (ebf, lo)
                  next_emit[r] = s_ + 1

              for b in range(nblk):
                mb = int(m[b].sum())
                # prefetch: emit up to (last segment consumed this block) + LOOKAHEAD
                for r in range(NR):
                    if m[b, r]:
                        s_end = (cursor[r] + int(m[b, r]) - 1 - int(Cr[r])) // SEG
                        while next_emit[r] <= min(s_end + LOOKAHEAD, n_seg[r] - 1):
                            emit_seg(r)
                o_sb = out_pool.tile([P, D], mybir.dt.float32)
                if mb == 0:
                    nc.vector.memset(o_sb[:], 0.0)
                    nc.sync.dma_start(
                        out=out_t[b * P : (b + 1) * P, :], in_=o_sb[:]
                    )
                    continue
                psum_tile = psum_pool.tile([P, D], dtype=mybir.dt.float32, space="PSUM")
                t = 0
                for r in range(NR):
                    for _j in range(int(m[b, r])):
                        c = cursor[r]
                        s_ = (c - int(Cr[r])) // SEG
                        ebf, lo = seg_tiles[r][s_]
                        if s_ - 1 in seg_tiles[r]:
                            del seg_tiles[r][s_ - 1]
                        if ablate in ("gather", "gconv"):
                            t += 1
                            q += 1
                            cursor[r] += 1
                            continue
                        if ablate not in ("pe", "nodve"):
                            oh_val = make_oh(q)
                        if ablate != "dve":
                            nc.tensor.matmul(
                                out=psum_tile[:],
                                lhsT=iota_sb[:]
                                if ablate in ("pe", "nodve")
                                else oh_val[:],
                                rhs=ebf[:, c - lo, :],
                                start=(t == 0),
                                stop=(t == mb - 1),
                            )
                        t += 1
                        q += 1
                        cursor[r] += 1
                if ablate in ("gather", "gconv", "dve"):
                    continue
                nc.scalar.copy(out=o_sb[:], in_=psum_tile[:])
                nc.sync.dma_start(out=out_t[b * P : (b + 1) * P, :], in_=o_sb[:])
    nc.compile()
    return nc


def _prepare(rows, cols, vals, n_nodes):
    """Host-side edge sort + chunk schedule. Returns (m, per-core arrays)."""
    npc = n_nodes // N_CORES
    nblk = (npc + P - 1) // P
    rb = -(-n_nodes // NR)

    rows = np.asarray(rows, dtype=np.int64)
    cols = np.asarray(cols, dtype=np.int64)
    vals = np.asarray(vals, dtype=np.float32)
    E = rows.shape[0]

    core = rows // npc
    bl = (rows % npc) // P
    rng = cols // rb
    rrow = (rows % npc - bl * P).astype(np.float32)
    G = N_CORES * nblk * NR
    key = (core * nblk + bl) * NR + rng
    order = np.argsort(key, kind="stable")
    ks = key[order]
    cs = cols[order]
    vs = vals[order]
    rs = rrow[order]

    cnt = np.bincount(key, minlength=G).reshape(N_CORES, nblk, NR)
    m = -(-cnt.max(axis=0) // P)  # [nblk, NR] ceil
    Q = int(m.sum())
    if Q == 0:
        return m, None

    # chunk bases
    Mr = m.sum(axis=0)
    Cr = np.concatenate([[0], np.cumsum(Mr)]).astype(np.int64)  # range-major
    cbase = np.zeros((nblk, NR), np.int64)  # range-major chunk idx of (b, r)
    for r in range(NR):
        cbase[:, r] = Cr[r] + np.concatenate([[0], np.cumsum(m[:-1, r])])
    qbase = (
        np.concatenate([[0], np.cumsum(m.reshape(-1))])[:-1]
        .reshape(nblk, NR)
        .astype(np.int64)
    )  # block-major

    # per-edge destination slots
    gstart = np.concatenate([[0], np.cumsum(cnt.reshape(-1))])[:-1]
    o = np.arange(E, dtype=np.int64) - gstart[ks]
    k_e = ks // (nblk * NR)
    b_e = (ks // NR) % nblk
    r_e = ks % NR

    idx_rm = np.zeros((N_CORES, Q * P), np.int16)
    val_bm = np.zeros((N_CORES, Q * P), np.float32)
    rrow_bm = np.zeros((N_CORES, Q * P), np.float32)
    dst_rm = k_e * (Q * P) + cbase[b_e, r_e] * P + o
    dst_bm = k_e * (Q * P) + qbase[b_e, r_e] * P + o
    idx_rm.reshape(-1)[dst_rm] = (cs - rb * r_e).astype(np.int16)
    val_bm.reshape(-1)[dst_bm] = vs
    rrow_bm.reshape(-1)[dst_bm] = rs

    # device layouts
    # gather idx stream wrapped: position i -> [i % 16, i // 16], replicated x8
    idx_w = np.ascontiguousarray(
        np.tile(idx_rm.reshape(N_CORES, Q * 8, 16).transpose(0, 2, 1), (1, 8, 1))
    )
    # vals/rrow: [P, Q] with slot (p, q) = edge q*128+p (bf16 DVE streams)
    from ml_dtypes import bfloat16

    vals_d = np.ascontiguousarray(val_bm.reshape(N_CORES, Q, P).transpose(0, 2, 1))
    rrow_d = np.ascontiguousarray(rrow_bm.reshape(N_CORES, Q, P).transpose(0, 2, 1))
    return m, (idx_w, vals_d, rrow_d)


def _kernel_impl(rows, cols, edge_vals, embeds, n_nodes, trace=False):
    from concourse.bass_utils import run_bass_kernel_spmd

    embeds = np.ascontiguousarray(np.asarray(embeds), dtype=np.float32)
    npc = n_nodes // N_CORES
    assert npc * N_CORES == n_nodes

    m, arrs = _prepare(rows, cols, edge_vals, n_nodes)
    if arrs is None:
        return np.zeros((n_nodes, D), np.float32)
    from ml_dtypes import bfloat16

    idx_w, vals_d, rrow_d = arrs
    iota = np.ascontiguousarray(np.tile(np.arange(P, dtype=bfloat16), (P, 1)))

    nc = _build_program(m, n_nodes)
    rb = -(-n_nodes // NR)
    emb_slices = {
        f"embeds_r{r}": np.ascontiguousarray(
            embeds[rb * r : min(rb * (r + 1), n_nodes)]
        )
        for r in range(NR)
    }
    in_maps = [
        {
            **emb_slices,
            "idx_w": idx_w[k],
            "vals_p": vals_d[k],
            "rrow_p": rrow_d[k],
            "iota": iota,
        }
        for k in range(N_CORES)
    ]
    global _LAST
    _LAST = (nc, in_maps)
    r = run_bass_kernel_spmd(nc, in_maps, list(range(N_CORES)), trace=trace)
    out = np.concatenate(
        [r.results[k]["out"][:npc] for k in range(N_CORES)], axis=0
    ).astype(np.float32)
    if trace:
        return out, r
    return out


_LAST = None


def kernel(rows, cols, edge_vals, embeds):
    return _kernel_impl(rows, cols, edge_vals, embeds, N_NODES)



# revision 11
# speedup vs baseline: 1.3260x; 1.3260x over previous
"""GCN layer (SpMM) Bass kernel for 8 trn2 NeuronCores.

out[i] = sum_{e: rows[e]==i} edge_vals[e] * embeds[cols[e]]
N=100000 nodes, E=1000000 edges, D=64 features.

Strategy: host sorts edges by destination row; nodes split into 8
contiguous ranges (12500/core, disjoint outputs -> no collectives).
Per core, output rows are processed in 128-row blocks. Edge embeddings
are fetched with batched SWDGE dma_gather (InstDMAGatherAnt): int16
indices limit a gather to a 32k-row window, so columns are partitioned
into 4 ranges of 25000 rows and each (block, range) group is padded to
whole chunks of 128 edges. Gathers run in 14-chunk segments
(1792 idxs, single_packet=False — >=1152 idxs with single_packet=True
kills the device — round-robin over 4 SWDGE queues, prefetched
LOOKAHEAD segments ahead of consumption). Per 16-chunk strip the DVE
builds val-scaled one-hots with two batched broadcast tensor_tensor ops
(bf16): oh[p, j, r] = (iota[r]==rrow[p,j]) * val[p,j]. Per 14-chunk
segment the scalar engine converts gathered embeddings fp32->bf16 in
one op. Per chunk, one bf16 matmul accumulates psum[r,:] += oh_j.T @
embs[p,:]. After a block's chunks, PSUM is copied to SBUF and DMA'd to
the contiguous output rows. Device time ~0.3 ms/exec (the gather is at
the random-256B HBM roofline ~140 GB/s/core); measured wall time is
dominated by the ~30-70 ms axon dispatch floor.
"""

import sys

import numpy as np

if "/opt/trn_rl_repo" not in sys.path:
    sys.path.insert(0, "/opt/trn_rl_repo")

N_NODES = 100000
D = 64
P = 128
N_CORES = 8
NR = 4  # column ranges (int16 gather index limit)
SEG = 15  # chunks per dma_gather (SWDGE ring limit: num_idxs/16+1 <= 128)


def _build_program(m, n_nodes, repeats=1, ablate=None):
    """m: [NBLK, NR] chunks per (block, range)."""
    import concourse.bacc as bacc
    import concourse.tile as tile
    from concourse import mybir

    npc = n_nodes // N_CORES
    nblk = (npc + P - 1) // P
    rb = -(-n_nodes // NR)  # range width
    Q = int(m.sum())
    # range-major chunk base: cbase[r] = start of range r's chunk span
    Mr = m.sum(axis=0)  # [NR]
    Cr = np.concatenate([[0], np.cumsum(Mr)]).astype(int)

    nc = bacc.Bacc(
        "TRN2",
        target_bir_lowering=False,
        debug=False,
        num_devices=N_CORES,
        num_swdge_queues=4,
    )
    # 4 separate range tensors: HW mis-applies large AP offsets on the
    # dma_gather source (sim honors them; >~256KB offsets read wrong data)
    emb_rt = [
        nc.dram_tensor(
            f"embeds_r{r}",
            [min(rb * (r + 1), n_nodes) - rb * r, D],
            mybir.dt.float32,
            kind="ExternalInput",
        )
        for r in range(NR)
    ]
    idx_t = nc.dram_tensor("idx_w", [P, Q * 8], mybir.dt.int16, kind="ExternalInput")
    vals_t = nc.dram_tensor("vals_r", [P, Q], mybir.dt.bfloat16, kind="ExternalInput")
    rrow_t = nc.dram_tensor("rrow_p", [P, Q], mybir.dt.bfloat16, kind="ExternalInput")
    iota_t = nc.dram_tensor("iota", [P, P], mybir.dt.bfloat16, kind="ExternalInput")
    out_t = nc.dram_tensor(
        "out", [nblk * P, D], mybir.dt.float32, kind="ExternalOutput"
    )

    import os as _os
    _ohb = int(_os.environ.get("OH_BUFS", "6"))
    _embb = int(_os.environ.get("EMB_BUFS", "5"))
    _ebfb = int(_os.environ.get("EBF_BUFS", "6"))
    with tile.TileContext(nc) as tc:
        with (
            tc.tile_pool(name="static", bufs=1) as static_pool,
            tc.tile_pool(name="emb0", bufs=_embb) as ep0,
            tc.tile_pool(name="emb1", bufs=_embb) as ep1,
            tc.tile_pool(name="emb2", bufs=_embb) as ep2,
            tc.tile_pool(name="emb3", bufs=_embb) as ep3,
            tc.tile_pool(name="ebf0", bufs=_ebfb) as bp0,
            tc.tile_pool(name="ebf1", bufs=_ebfb) as bp1,
            tc.tile_pool(name="ebf2", bufs=_ebfb) as bp2,
            tc.tile_pool(name="ebf3", bufs=_ebfb) as bp3,
            tc.tile_pool(name="ohv", bufs=_ohb) as ohv_pool,
            tc.tile_pool(name="outp", bufs=4) as out_pool,
            tc.tile_pool(name="psum", bufs=8, space="PSUM") as psum_pool,
        ):
            embp = [ep0, ep1, ep2, ep3]
            ebfp = [bp0, bp1, bp2, bp3]
            idx_sb = static_pool.tile([P, Q * 8], mybir.dt.int16)
            vals_sb = static_pool.tile([P, Q], mybir.dt.bfloat16)
            rrow_sb = static_pool.tile([P, Q], mybir.dt.bfloat16)
            iota_sb = static_pool.tile([P, P], mybir.dt.bfloat16)
            nc.sync.dma_start(out=idx_sb[:], in_=idx_t[:])
            nc.sync.dma_start(out=vals_sb[:], in_=vals_t[:])
            nc.sync.dma_start(out=rrow_sb[:], in_=rrow_t[:])
            nc.sync.dma_start(out=iota_sb[:], in_=iota_t[:])

            import os as _os
            LOOKAHEAD = int(_os.environ.get("LOOKAHEAD", "3"))
            for _rep in range(repeats):
              cursor = [int(Cr[r]) for r in range(NR)]
              seg_tiles = [dict() for _ in range(NR)]  # s -> (ebf, lo)
              next_emit = [0] * NR
              n_seg = [
                  -(-int(Mr[r]) // SEG) if int(Mr[r]) else 0 for r in range(NR)
              ]
              q = 0
              n_gather = 0
              strip = {}  # q0 -> oh_eq tile
              CSTRIP = int(_os.environ.get("CSTRIP", "16"))

              def get_strip(q0):
                  # oh_eq[p, j, r] = (rrow[p, q0+j] == iota[r]); 0/1 one-hot
                  # WITHOUT the edge value — the value is folded into the
                  # gathered-embedding tile per segment (64-wide per edge
                  # instead of 128-wide here), halving DVE element work.
                  if q0 in strip:
                      return strip[q0]
                  C = min(CSTRIP, Q - q0)
                  iota_b = iota_sb[:, :].rearrange(
                      "p (one d) -> p one d", one=1
                  ).to_broadcast([P, C, P])
                  rrow_b = rrow_sb[:, q0 : q0 + C].to_broadcast([P, C, P])
                  oh_eq = ohv_pool.tile([P, C, P], mybir.dt.bfloat16)
                  nc.vector.tensor_tensor(
                      out=oh_eq[:], in0=rrow_b, in1=iota_b,
                      op=mybir.AluOpType.is_equal,
                  )
                  strip.clear()
                  strip[q0] = oh_eq
                  return oh_eq

              def emit_seg(r):
                  nonlocal n_gather
                  s_ = next_emit[r]
                  lo = int(Cr[r]) + s_ * SEG
                  hi = min(lo + SEG, int(Cr[r + 1]))
                  L = hi - lo
                  emb = embp[r].tile([P, L, D], mybir.dt.float32)
                  if ablate not in ("comp", "dve", "pe"):
                      nc.gpsimd.dma_gather(
                          emb[:],
                          emb_rt[r][:],
                          idx_sb[:, lo * 8 : hi * 8],
                          L * P,
                          L * P,
                          D,
                          single_packet=False,
                          queue_num=n_gather % 4,
                      )
                      n_gather += 1
                  ebf = ebfp[r].tile([P, L, D], mybir.dt.bfloat16)
                  if ablate not in ("gather", "comp", "dve", "pe"):
                      nc.scalar.copy(out=ebf[:], in_=emb[:])
                      # fold the edge value into the gathered rows:
                      # ebf[p, j, :] *= val[p, lo+j] (range-major stream)
                      nc.vector.tensor_tensor(
                          out=ebf[:], in0=ebf[:],
                          in1=vals_sb[:, lo:hi].to_broadcast([P, L, D]),
                          op=mybir.AluOpType.mult,
                      )
                  elif ablate in ("comp", "dve", "pe"):
                      nc.gpsimd.memzero(ebf[:])
                  seg_tiles[r][s_] = (ebf, lo)
                  next_emit[r] = s_ + 1

              for b in range(nblk):
                mb = int(m[b].sum())
                # prefetch: emit up to (last segment consumed this block) + LOOKAHEAD
                for r in range(NR):
                    if m[b, r]:
                        s_end = (cursor[r] + int(m[b, r]) - 1 - int(Cr[r])) // SEG
                        while next_emit[r] <= min(s_end + LOOKAHEAD, n_seg[r] - 1):
                            emit_seg(r)
                o_sb = out_pool.tile([P, D], mybir.dt.float32)
                if mb == 0:
                    nc.vector.memset(o_sb[:], 0.0)
                    nc.sync.dma_start(
                        out=out_t[b * P : (b + 1) * P, :], in_=o_sb[:]
                    )
                    continue
                psum_tile = psum_pool.tile([P, D], dtype=mybir.dt.float32, space="PSUM")
                t = 0
                for r in range(NR):
                    for _j in range(int(m[b, r])):
                        c = cursor[r]
                        s_ = (c - int(Cr[r])) // SEG
                        ebf, lo = seg_tiles[r][s_]
                        if s_ - 1 in seg_tiles[r]:
                            del seg_tiles[r][s_ - 1]
                        if ablate in ("gather", "gconv"):
                            t += 1
                            q += 1
                            cursor[r] += 1
                            continue
                        if ablate not in ("pe", "nodve"):
                            q0 = (q // CSTRIP) * CSTRIP
                            oh_eq = get_strip(q0)
                        if ablate != "dve":
                            nc.tensor.matmul(
                                out=psum_tile[:],
                                lhsT=iota_sb[:]
                                if ablate in ("pe", "nodve")
                                else oh_eq[:, q - q0, :],
                                rhs=ebf[:, c - lo, :],
                                start=(t == 0),
                                stop=(t == mb - 1),
                            )
                        t += 1
                        q += 1
                        cursor[r] += 1
                if ablate in ("gather", "gconv", "dve"):
                    continue
                nc.scalar.copy(out=o_sb[:], in_=psum_tile[:])
                nc.sync.dma_start(out=out_t[b * P : (b + 1) * P, :], in_=o_sb[:])
    nc.compile()
    return nc


def _prepare(rows, cols, vals, n_nodes):
    """Host-side edge sort + chunk schedule. Returns (m, per-core arrays)."""
    npc = n_nodes // N_CORES
    nblk = (npc + P - 1) // P
    rb = -(-n_nodes // NR)

    rows = np.asarray(rows, dtype=np.int64)
    cols = np.asarray(cols, dtype=np.int64)
    vals = np.asarray(vals, dtype=np.float32)
    E = rows.shape[0]

    core = rows // npc
    bl = (rows % npc) // P
    rng = cols // rb
    rrow = (rows % npc - bl * P).astype(np.float32)
    G = N_CORES * nblk * NR
    key = (core * nblk + bl) * NR + rng
    order = np.argsort(key, kind="stable")
    ks = key[order]
    cs = cols[order]
    vs = vals[order]
    rs = rrow[order]

    cnt = np.bincount(key, minlength=G).reshape(N_CORES, nblk, NR)
    m = -(-cnt.max(axis=0) // P)  # [nblk, NR] ceil
    Q = int(m.sum())
    if Q == 0:
        return m, None

    # chunk bases
    Mr = m.sum(axis=0)
    Cr = np.concatenate([[0], np.cumsum(Mr)]).astype(np.int64)  # range-major
    cbase = np.zeros((nblk, NR), np.int64)  # range-major chunk idx of (b, r)
    for r in range(NR):
        cbase[:, r] = Cr[r] + np.concatenate([[0], np.cumsum(m[:-1, r])])
    qbase = (
        np.concatenate([[0], np.cumsum(m.reshape(-1))])[:-1]
        .reshape(nblk, NR)
        .astype(np.int64)
    )  # block-major

    # per-edge destination slots
    gstart = np.concatenate([[0], np.cumsum(cnt.reshape(-1))])[:-1]
    o = np.arange(E, dtype=np.int64) - gstart[ks]
    k_e = ks // (nblk * NR)
    b_e = (ks // NR) % nblk
    r_e = ks % NR

    idx_rm = np.zeros((N_CORES, Q * P), np.int16)
    val_rm = np.zeros((N_CORES, Q * P), np.float32)
    rrow_bm = np.zeros((N_CORES, Q * P), np.float32)
    dst_rm = k_e * (Q * P) + cbase[b_e, r_e] * P + o
    dst_bm = k_e * (Q * P) + qbase[b_e, r_e] * P + o
    idx_rm.reshape(-1)[dst_rm] = (cs - rb * r_e).astype(np.int16)
    # vals are RANGE-major (aligned with gather segments for the ebf scale);
    # rrow stays block-major (aligned with the oh strip / matmul order).
    val_rm.reshape(-1)[dst_rm] = vs
    rrow_bm.reshape(-1)[dst_bm] = rs

    # device layouts
    # gather idx stream wrapped: position i -> [i % 16, i // 16], replicated x8
    idx_w = np.ascontiguousarray(
        np.tile(idx_rm.reshape(N_CORES, Q * 8, 16).transpose(0, 2, 1), (1, 8, 1))
    )
    # vals/rrow: [P, Q] with slot (p, q) = edge q*128+p (bf16 DVE streams)
    from ml_dtypes import bfloat16

    vals_d = np.ascontiguousarray(
        val_rm.reshape(N_CORES, Q, P).transpose(0, 2, 1).astype(bfloat16)
    )
    rrow_d = np.ascontiguousarray(
        rrow_bm.reshape(N_CORES, Q, P).transpose(0, 2, 1).astype(bfloat16)
    )
    return m, (idx_w, vals_d, rrow_d)


def _kernel_impl(rows, cols, edge_vals, embeds, n_nodes, trace=False):
    from concourse.bass_utils import run_bass_kernel_spmd

    embeds = np.ascontiguousarray(np.asarray(embeds), dtype=np.float32)
    npc = n_nodes // N_CORES
    assert npc * N_CORES == n_nodes

    m, arrs = _prepare(rows, cols, edge_vals, n_nodes)
    if arrs is None:
        return np.zeros((n_nodes, D), np.float32)
    from ml_dtypes import bfloat16

    idx_w, vals_d, rrow_d = arrs
    iota = np.ascontiguousarray(np.tile(np.arange(P, dtype=bfloat16), (P, 1)))

    nc = _build_program(m, n_nodes)
    rb = -(-n_nodes // NR)
    emb_slices = {
        f"embeds_r{r}": np.ascontiguousarray(
            embeds[rb * r : min(rb * (r + 1), n_nodes)]
        )
        for r in range(NR)
    }
    in_maps = [
        {
            **emb_slices,
            "idx_w": idx_w[k],
            "vals_r": vals_d[k],
            "rrow_p": rrow_d[k],
            "iota": iota,
        }
        for k in range(N_CORES)
    ]
    global _LAST
    _LAST = (nc, in_maps)
    r = run_bass_kernel_spmd(nc, in_maps, list(range(N_CORES)), trace=trace)
    out = np.concatenate(
        [r.results[k]["out"][:npc] for k in range(N_CORES)], axis=0
    ).astype(np.float32)
    if trace:
        return out, r
    return out


_LAST = None


def kernel(rows, cols, edge_vals, embeds):
    return _kernel_impl(rows, cols, edge_vals, embeds, N_NODES)



# revision 12
# speedup vs baseline: 23.9805x; 18.0848x over previous
"""GCN layer (SpMM) Bass kernel for 8 trn2 NeuronCores — V3.

out[i] = sum_{e: rows[e]==i} edge_vals[e] * embeds[cols[e]]
N=100000 nodes, E=1000000 edges, D=64 features.

V3 vs V2: edge slots are packed per (block, range) cell at 16-slot
granularity (not whole 128-slot chunks), so chunks can cross cell
boundaries; a chunk shared by two dst blocks gets one matmul per block
with a host-masked oh column (val=0 outside the block's slot span).
Dst-row blocks of 128 are global (g = row // 128) and assigned to
(core, local_b) slots by a greedy 4-vector matcher that groups blocks
with similar per-range edge counts, shrinking the shared SPMD span
(max over cores). Cuts gather padding from ~20% to ~7%.
"""

import sys

import numpy as np

if "/opt/trn_rl_repo" not in sys.path:
    sys.path.insert(0, "/opt/trn_rl_repo")

N_NODES = 100000
D = 64
P = 128
N_CORES = 8
NR = 4  # column ranges (int16 gather index limit)
SEG = 15  # chunks per dma_gather (SWDGE ring limit: num_idxs/16+1 <= 128)
GRAN = 16  # slot granularity of cell spans (idx stream wrap granularity)


def _schedule(span):
    """Derive the shared chunk grid + matmul schedule from cell spans.

    span: [nblk, NR] slots per cell (GRAN-granular).
    Returns dict with slot bases, chunk counts, and per-block entries.
    """
    nblk = span.shape[0]
    S_r = span.sum(axis=0)  # slots per range
    nchunk = -(-S_r // P)  # chunks per range
    cbase = np.concatenate([[0], np.cumsum(nchunk)]).astype(int)  # chunk base
    sbase = cbase * P  # slot base per range
    # slot0[b, r]: global slot where cell (b, r) starts
    slot0 = np.zeros((nblk, NR), np.int64)
    for r in range(NR):
        slot0[:, r] = sbase[r] + np.concatenate([[0], np.cumsum(span[:-1, r])])
    Q = int(nchunk.sum())
    # matmul entries, block-major: (b, r, chunk c) -> column j
    entries = []  # per block: list of (r, c)
    for b in range(nblk):
        ent = []
        for r in range(NR):
            if span[b, r] == 0:
                continue
            s0, s1 = slot0[b, r], slot0[b, r] + span[b, r] - 1
            for c in range(int(s0 // P), int(s1 // P) + 1):
                ent.append((r, c))
        entries.append(ent)
    Qm = sum(len(e) for e in entries)
    return dict(
        span=span, slot0=slot0, nchunk=nchunk, cbase=cbase, Q=Q, Qm=Qm,
        entries=entries, nblk=nblk,
    )


def _build_program(sched, n_nodes, ablate=None):
    import os as _os

    import concourse.bacc as bacc
    import concourse.tile as tile
    from concourse import mybir

    rb = -(-n_nodes // NR)
    nblk = sched["nblk"]
    Q, Qm = sched["Q"], sched["Qm"]
    nchunk, cbase = sched["nchunk"], sched["cbase"]
    entries = sched["entries"]

    nc = bacc.Bacc(
        "TRN2",
        target_bir_lowering=False,
        debug=False,
        num_devices=N_CORES,
        num_swdge_queues=4,
    )
    emb_rt = [
        nc.dram_tensor(
            f"embeds_r{r}",
            [min(rb * (r + 1), n_nodes) - rb * r, D],
            mybir.dt.float32,
            kind="ExternalInput",
        )
        for r in range(NR)
    ]
    idx_t = nc.dram_tensor("idx_w", [P, Q * 8], mybir.dt.int16, kind="ExternalInput")
    vals_t = nc.dram_tensor("vals_r", [P, Q], mybir.dt.bfloat16, kind="ExternalInput")
    rrow_t = nc.dram_tensor("rrow_p", [P, Qm], mybir.dt.bfloat16, kind="ExternalInput")
    iota_t = nc.dram_tensor("iota", [P, P], mybir.dt.bfloat16, kind="ExternalInput")
    out_t = nc.dram_tensor(
        "out", [nblk * P, D], mybir.dt.float32, kind="ExternalOutput"
    )

    _ohb = int(_os.environ.get("OH_BUFS", "6"))
    _embb = int(_os.environ.get("EMB_BUFS", "5"))
    _ebfb = int(_os.environ.get("EBF_BUFS", "6"))
    LOOKAHEAD = int(_os.environ.get("LOOKAHEAD", "3"))
    with tile.TileContext(nc) as tc:
        with (
            tc.tile_pool(name="static", bufs=1) as static_pool,
            tc.tile_pool(name="emb0", bufs=_embb) as ep0,
            tc.tile_pool(name="emb1", bufs=_embb) as ep1,
            tc.tile_pool(name="emb2", bufs=_embb) as ep2,
            tc.tile_pool(name="emb3", bufs=_embb) as ep3,
            tc.tile_pool(name="ebf0", bufs=_ebfb) as bp0,
            tc.tile_pool(name="ebf1", bufs=_ebfb) as bp1,
            tc.tile_pool(name="ebf2", bufs=_ebfb) as bp2,
            tc.tile_pool(name="ebf3", bufs=_ebfb) as bp3,
            tc.tile_pool(name="ohv", bufs=_ohb) as ohv_pool,
            tc.tile_pool(name="outp", bufs=4) as out_pool,
            tc.tile_pool(name="psum", bufs=8, space="PSUM") as psum_pool,
        ):
            embp = [ep0, ep1, ep2, ep3]
            ebfp = [bp0, bp1, bp2, bp3]
            idx_sb = static_pool.tile([P, Q * 8], mybir.dt.int16)
            vals_sb = static_pool.tile([P, Q], mybir.dt.bfloat16)
            rrow_sb = static_pool.tile([P, Qm], mybir.dt.bfloat16)
            iota_sb = static_pool.tile([P, P], mybir.dt.bfloat16)
            nc.sync.dma_start(out=idx_sb[:], in_=idx_t[:])
            nc.sync.dma_start(out=vals_sb[:], in_=vals_t[:])
            nc.sync.dma_start(out=rrow_sb[:], in_=rrow_t[:])
            nc.sync.dma_start(out=iota_sb[:], in_=iota_t[:])

            seg_tiles = [dict() for _ in range(NR)]  # s -> ebf tile
            next_emit = [0] * NR
            n_seg = [-(-int(nchunk[r]) // SEG) if nchunk[r] else 0 for r in range(NR)]
            n_gather = 0
            j = 0  # oh column cursor (block-major entry order)

            strip = {}  # j0 -> oh_eq tile
            CSTRIP = int(_os.environ.get("CSTRIP", "16"))

            def get_strip(j0):
                # oh_eq[p, j, r] = (rrow[p, j0+j] == iota[r]); masked/pad
                # slots carry rrow=-1 (misses every column). Edge values are
                # folded into the gathered tiles per segment instead.
                if j0 in strip:
                    return strip[j0]
                C = min(CSTRIP, Qm - j0)
                iota_b = iota_sb[:, :].rearrange(
                    "p (one d) -> p one d", one=1
                ).to_broadcast([P, C, P])
                rrow_b = rrow_sb[:, j0 : j0 + C].to_broadcast([P, C, P])
                oh_eq = ohv_pool.tile([P, C, P], mybir.dt.bfloat16)
                nc.vector.tensor_tensor(
                    out=oh_eq[:], in0=rrow_b, in1=iota_b,
                    op=mybir.AluOpType.is_equal,
                )
                strip.clear()
                strip[j0] = oh_eq
                return oh_eq

            def emit_seg(r):
                nonlocal n_gather
                s_ = next_emit[r]
                lo = int(cbase[r]) + s_ * SEG  # global chunk number
                hi = min(lo + SEG, int(cbase[r + 1]))
                L = hi - lo
                emb = embp[r].tile([P, L, D], mybir.dt.float32)
                if ablate not in ("comp",):
                    nc.gpsimd.dma_gather(
                        emb[:],
                        emb_rt[r][:],
                        idx_sb[:, lo * 8 : hi * 8],
                        L * P,
                        L * P,
                        D,
                        single_packet=False,
                        queue_num=n_gather % 4,
                    )
                    n_gather += 1
                ebf = ebfp[r].tile([P, L, D], mybir.dt.bfloat16)
                if ablate not in ("gather", "comp"):
                    nc.scalar.copy(out=ebf[:], in_=emb[:])
                    # fold edge values into the gathered rows (vals_r is
                    # slot/chunk-aligned, same order as the gather stream)
                    nc.vector.tensor_tensor(
                        out=ebf[:], in0=ebf[:],
                        in1=vals_sb[:, lo:hi].to_broadcast([P, L, D]),
                        op=mybir.AluOpType.mult,
                    )
                elif ablate == "comp":
                    nc.gpsimd.memzero(ebf[:])
                seg_tiles[r][s_] = ebf
                next_emit[r] = s_ + 1

            for b in range(nblk):
                ent = entries[b]
                # prefetch segments LOOKAHEAD ahead of this block's last chunk
                for r in range(NR):
                    cmax = max((c for (r_, c) in ent if r_ == r), default=None)
                    if cmax is None:
                        continue
                    s_end = (cmax - int(cbase[r])) // SEG
                    while next_emit[r] <= min(s_end + LOOKAHEAD, n_seg[r] - 1):
                        emit_seg(r)
                o_sb = out_pool.tile([P, D], mybir.dt.float32)
                if not ent:
                    nc.vector.memset(o_sb[:], 0.0)
                    nc.sync.dma_start(
                        out=out_t[b * P : (b + 1) * P, :], in_=o_sb[:]
                    )
                    continue
                psum_tile = psum_pool.tile([P, D], dtype=mybir.dt.float32, space="PSUM")
                mb = len(ent)
                for t, (r, c) in enumerate(ent):
                    rel = c - int(cbase[r])
                    s_ = rel // SEG
                    ebf = seg_tiles[r][s_]
                    if s_ - 1 in seg_tiles[r] and rel % SEG > 0:
                        del seg_tiles[r][s_ - 1]
                    if ablate in ("gather",):
                        j += 1
                        continue
                    j0 = (j // CSTRIP) * CSTRIP
                    oh_eq = get_strip(j0)
                    nc.tensor.matmul(
                        out=psum_tile[:],
                        lhsT=oh_eq[:, j - j0, :],
                        rhs=ebf[:, rel % SEG, :],
                        start=(t == 0),
                        stop=(t == mb - 1),
                    )
                    j += 1
                if ablate in ("gather",):
                    continue
                nc.scalar.copy(out=o_sb[:], in_=psum_tile[:])
                nc.sync.dma_start(out=out_t[b * P : (b + 1) * P, :], in_=o_sb[:])
    nc.compile()
    return nc


def _prepare(rows, cols, vals, n_nodes):
    """Host-side: block assignment, packed slot layout, oh columns.

    Returns (sched, block_of[core, local_b] (-1 = virtual),
             (idx_w, vals_d, rrow_d))."""
    rows = np.asarray(rows, dtype=np.int64)
    cols = np.asarray(cols, dtype=np.int64)
    vals = np.asarray(vals, dtype=np.float32)
    E = rows.shape[0]
    rb = -(-n_nodes // NR)
    nbg = -(-n_nodes // P)  # global 128-row blocks
    nblk = -(-nbg // N_CORES)

    g = rows // P
    rrow = (rows % P).astype(np.float32)
    rng = cols // rb

    cnt_g = np.zeros((nbg, NR), np.int64)
    np.add.at(cnt_g, (g, rng), 1)

    # greedy matcher: place blocks (desc total) into nblk groups of <=8,
    # minimizing the increase of sum_r max
    order = np.argsort(-cnt_g.sum(axis=1), kind="stable")
    gmax = np.zeros((nblk, NR), np.int64)
    gfill = np.zeros(nblk, np.int64)
    group_members = [[] for _ in range(nblk)]
    for blk in order:
        c = cnt_g[blk]
        inc = np.maximum(gmax, c).sum(axis=1) - gmax.sum(axis=1)
        inc[gfill >= N_CORES] = 1 << 60
        tgt = int(np.argmin(inc))
        gmax[tgt] = np.maximum(gmax[tgt], c)
        gfill[tgt] += 1
        group_members[tgt].append(int(blk))

    block_of = -np.ones((N_CORES, nblk), np.int64)
    core_of = np.zeros(nbg, np.int64)
    lb_of = np.zeros(nbg, np.int64)
    for b in range(nblk):
        for k, blk in enumerate(group_members[b]):
            block_of[k, b] = blk
            core_of[blk] = k
            lb_of[blk] = b

    span = -(-gmax // GRAN) * GRAN  # [nblk, NR]
    sched = _schedule(span)
    Q, Qm = sched["Q"], sched["Qm"]
    slot0 = sched["slot0"]
    entries = sched["entries"]

    # column base per (b, r): j index of entry (b, r, c0)
    jbase = {}
    jc = 0
    for b in range(nblk):
        seen = {}
        for (r, c) in entries[b]:
            if r not in seen:
                seen[r] = jc
            jc += 1
        for r, v in seen.items():
            jbase[(b, r)] = v

    # per-edge placement
    k_e = core_of[g]
    b_e = lb_of[g]
    key = (k_e * nblk + b_e) * NR + rng
    order_e = np.argsort(key, kind="stable")
    ks = key[order_e]
    cs = cols[order_e]
    vs = vals[order_e]
    rs = rrow[order_e]
    gstart = np.concatenate([[0], np.cumsum(np.bincount(ks, minlength=N_CORES * nblk * NR))])[:-1]
    o = np.arange(E, dtype=np.int64) - gstart[ks]
    k_s = ks // (nblk * NR)
    b_s = (ks // NR) % nblk
    r_s = ks % NR

    slot_e = slot0[b_s, r_s] + o  # global slot
    chunk_e = slot_e // P
    p_e = slot_e % P
    c0 = slot0[b_s, r_s] // P
    j_e = np.array([jbase[(int(b), int(r))] for b, r in zip(b_s, r_s)]) \
        if False else None
    # vectorized jbase lookup
    jbase_arr = np.full((nblk, NR), -1, np.int64)
    for (b, r), v in jbase.items():
        jbase_arr[b, r] = v
    j_e = jbase_arr[b_s, r_s] + (chunk_e - c0)
    assert (j_e >= 0).all() and (j_e < Qm).all()

    from ml_dtypes import bfloat16

    idx_rm = np.zeros((N_CORES, Q * P), np.int16)
    # vals are slot-aligned (range-major, same order as the gather stream);
    # pad slots keep val=0 so their gathered rows are zeroed before matmul.
    val_rm = np.zeros((N_CORES, Q * P), np.float32)
    # oh columns are block-major entries; slots outside the entry's block
    # span carry rrow=-1 (one-hot misses every dst row).
    rrow_bm = np.full((N_CORES, Qm * P), -1.0, np.float32)
    idx_rm.reshape(-1)[k_s * (Q * P) + slot_e] = (cs - rb * r_s).astype(np.int16)
    val_rm.reshape(-1)[k_s * (Q * P) + slot_e] = vs
    rrow_bm.reshape(-1)[k_s * (Qm * P) + j_e * P + p_e] = rs

    idx_w = np.ascontiguousarray(
        np.tile(idx_rm.reshape(N_CORES, Q * 8, 16).transpose(0, 2, 1), (1, 8, 1))
    )
    vals_d = np.ascontiguousarray(
        val_rm.reshape(N_CORES, Q, P).transpose(0, 2, 1).astype(bfloat16)
    )
    rrow_d = np.ascontiguousarray(
        rrow_bm.reshape(N_CORES, Qm, P).transpose(0, 2, 1).astype(bfloat16)
    )
    return sched, block_of, (idx_w, vals_d, rrow_d)


def _kernel_impl(rows, cols, edge_vals, embeds, n_nodes, trace=False, ablate=None):
    from concourse.bass_utils import run_bass_kernel_spmd
    from ml_dtypes import bfloat16

    embeds = np.ascontiguousarray(np.asarray(embeds), dtype=np.float32)
    rb = -(-n_nodes // NR)

    sched, block_of, arrs = _prepare(rows, cols, edge_vals, n_nodes)
    idx_w, vals_d, rrow_d = arrs
    iota = np.ascontiguousarray(np.tile(np.arange(P, dtype=bfloat16), (P, 1)))

    nc = _build_program(sched, n_nodes, ablate=ablate)
    emb_slices = {
        f"embeds_r{r}": np.ascontiguousarray(
            embeds[rb * r : min(rb * (r + 1), n_nodes)]
        )
        for r in range(NR)
    }
    in_maps = [
        {
            **emb_slices,
            "idx_w": idx_w[k],
            "vals_r": vals_d[k],
            "rrow_p": rrow_d[k],
            "iota": iota,
        }
        for k in range(N_CORES)
    ]
    global _LAST
    _LAST = (nc, in_maps)
    r = run_bass_kernel_spmd(nc, in_maps, list(range(N_CORES)), trace=trace)
    nblk = sched["nblk"]
    out = np.zeros((n_nodes, D), np.float32)
    for k in range(N_CORES):
        res = r.results[k]["out"]
        for b in range(nblk):
            blk = block_of[k, b]
            if blk < 0:
                continue
            lo = blk * P
            hi = min(lo + P, n_nodes)
            out[lo:hi] = res[b * P : b * P + (hi - lo)]
    if trace:
        return out, r
    return out


_LAST = None


def kernel(rows, cols, edge_vals, embeds):
    return _kernel_impl(rows, cols, edge_vals, embeds, N_NODES)
